# revision 4
# baseline (speedup 1.0000x reference)
"""DGCNN-style point-cloud classifier on 8 Trainium2 NeuronCores.

Data-parallel over the B=16 point-cloud axis: each of the 8 cores processes 2
clouds end-to-end (kNN -> EdgeConv1 -> kNN -> EdgeConv2 -> lin1 -> global max
pool -> head -> log_softmax).

Host-dispatch-optimized revision: the per-call wall clock is dominated by
shipping inputs over the PJRT tunnel, so the weights travel as ONE bf16 blob
sharded 1/8th per core and AllGather-ed on device (plus a small f32 blob for
biases/structural matrices), and the purely structural constants (iota, the
kNN diagonal mask, the self-index gather table) are generated on device with
iota/memset instructions.  Total upload: ~2.3 MB vs ~39 MB for the naive
per-core replication.  A dummy custom-DVE op keeps the per-call DVE table
generation on the process-level cache path.

Device-side ideas (unchanged from the baseline):
  * kNN top-20 per point via packed int32 keys (2^30 - d*S | neighbor index in
    the low 10 bits) extracted with DVE Max8 + MatchReplace (3+2 passes).
  * Neighbor gathers with GPSIMD ap_gather in a feature-major layout, which is
    exactly the transposed layout TensorE wants for the per-edge MLP.
  * EdgeConv2's single linear layer folds through the max-aggregation:
    out_i = pre_i + max_j q_j, so no per-edge GEMM at all.
  * GEMM weights are kept in bf16 (activations converted at layer inputs);
    the kNN distance pipeline stays fully fp32.
"""

import sys
import numpy as np
from functools import lru_cache

for _p in ("/opt/trn_rl_repo", "/root/.axon_site/_ro/trn_rl_repo"):
    if _p not in sys.path:
        sys.path.insert(0, _p)

import ml_dtypes
import concourse.bass as bass
import concourse.bacc as bacc
import concourse.mybir as mybir
import concourse.tile as tile
from concourse.bass_utils import run_bass_kernel_spmd

AF = mybir.ActivationFunctionType
ALU = mybir.AluOpType
DT = mybir.dt
F32 = DT.float32
BF16 = DT.bfloat16
I32 = DT.int32
I16 = DT.int16

N = 1024          # points per cloud
K = 20            # neighbors
NCORES = 8
CPC = 2           # clouds per core
NB = 8            # point blocks of 128 per cloud
E = K * 128       # edges per point block (2560)
NCH = 5           # 512-col chunks per point block of edges

SCALE1 = float(1 << 24)   # key scale for kNN1 (d range 127, resolution 2^-14)
SCALE2 = float(1 << 20)   # key scale for kNN2 (d range 2040, resolution 2^-10)
BIAS30 = float(1 << 30)

# ---- packed-blob layouts: (name, partitions, cols); flattened p-major ----
LAYOUT16 = [
    ("W1bb", 128, 128), ("W1cc", 128, 128), ("W2r2", 128, 128),
    ("PmQ2", 128, 128), ("Wl_a2", 128, 1024), ("Wl_b", 128, 1024),
    ("Wm1r", 128, 4096), ("Wm2r", 128, 1024), ("Wm3r", 128, 80),
    ("AmB", 3, 64), ("B3", 3, 64),
]
LAYOUT32 = [
    ("E1r", 128, 66), ("E2r", 128, 66), ("I64st", 128, 64), ("I40", 40, 40),
    ("E1p", 3, 5), ("E2p", 3, 5), ("b1a_c", 64, 1), ("b1bb", 128, 1),
    ("b1cc", 128, 1), ("b2c", 128, 1), ("blT2", 128, 16), ("bm1b", 128, 4),
    ("bm2b", 128, 2), ("bm3T", 40, 1),
]


def _offsets(layout):
    d, off = {}, 0
    for name, p, c in layout:
        d[name] = (p, c, off)
        off += p * c
    return d, off


L16, TOT16 = _offsets(LAYOUT16)
L32, TOT32 = _offsets(LAYOUT32)
SH16 = -(-TOT16 // 8)          # per-core shard elems (bf16)
SH32 = -(-TOT32 // 8)          # per-core shard elems (f32)


def _knn_block(nc, pool, psum_alloc, lhsT_A, rhs_B, scale, iota2d, diag2048,
               idx16_all, blk):
    """Top-20 neighbor indices for one 128-point block.

    lhsT_A: [Kc x 128] block slice of the augmented A operand.
    rhs_B:  [Kc x 1024] augmented B operand. psum = A.T@B = -d/2 per pair.
    Writes int16 indices into idx16_all[:, 20*blk : 20*(blk+1)].
    """
    ps = psum_alloc()
    nc.tensor.matmul(out=ps[:, 0:512], lhsT=lhsT_A,
                     rhs=rhs_B[:, 0:512], start=True, stop=True)
    nc.tensor.matmul(out=ps[:, 512:1024], lhsT=lhsT_A,
                     rhs=rhs_B[:, 512:1024], start=True, stop=True)
    keys = pool.tile([128, N], I32, tag="keys", name="keys")
    nc.scalar.activation(keys[:], ps[:], AF.Copy, bias=BIAS30, scale=scale)
    # clear low 10 bits, boost the diagonal (self) above everything, add index
    nc.vector.tensor_scalar(out=keys[:], in0=keys[:], scalar1=-1024,
                            scalar2=None, op0=ALU.bitwise_and)
    nc.vector.tensor_tensor(out=keys[:, 128 * blk:128 * (blk + 1)],
                            in0=keys[:, 128 * blk:128 * (blk + 1)],
                            in1=diag2048[:], op=ALU.add)
    nc.vector.tensor_tensor(out=keys[:], in0=keys[:], in1=iota2d[:],
                            op=ALU.bitwise_or)
    kf = keys[:].bitcast(F32)
    top = pool.tile([128, 24], F32, tag="top24", name="top24")
    nc.vector.max(out=top[:, 0:8], in_=kf)
    nc.vector.match_replace(out=kf, in_to_replace=top[:, 0:8], in_values=kf,
                            imm_value=0.0)
    nc.vector.max(out=top[:, 8:16], in_=kf)
    nc.vector.match_replace(out=kf, in_to_replace=top[:, 8:16], in_values=kf,
                            imm_value=0.0)
    nc.vector.max(out=top[:, 16:24], in_=kf)
    # col 0 is self; neighbor indices are the low 10 bits of cols 1..20
    idxs = pool.tile([128, K], I32, tag="idx32", name="idx32")
    nc.vector.tensor_scalar(out=idxs[:], in0=top[:, 1:21].bitcast(I32),
                            scalar1=1023, scalar2=None, op0=ALU.bitwise_and)
    nc.vector.tensor_copy(out=idx16_all[:, K * blk:K * (blk + 1)], in_=idxs[:])


def _fold_idx(nc, idx16_all, wrapped, ngroups_log2):
    """[128 x 160] per-point indices -> ap_gather wrapped layout [16 x 1280],
    then replicate across partition groups by doubling."""
    for b in range(8):
        src = idx16_all[16 * b:16 * (b + 1), :].rearrange("q (pb e) -> q pb e", e=K)
        dst = wrapped[0:16, :].rearrange("q (pb e b) -> q pb e b", e=K, b=8)[:, :, :, b]
        nc.sync.dma_start(out=dst, in_=src)
    for i in range(ngroups_log2):
        w = 16 << i
        nc.sync.dma_start(out=wrapped[w:2 * w, :], in_=wrapped[0:w, :])


def build_program():
    nc = bacc.Bacc("TRN2", target_bir_lowering=False, debug=False)

    def inp(name, shape, dtype=F32):
        return nc.dram_tensor(name, list(shape), dtype, kind="ExternalInput").ap()

    posT2 = inp("posT2", (CPC, 3, N))
    wsh16 = inp("wsh16", (1, SH16), BF16)
    wsh32 = inp("wsh32", (1, SH32), F32)
    out2 = nc.dram_tensor("out2", [CPC, 40], F32, kind="ExternalOutput").ap()

    with tile.TileContext(nc) as tc:
        _core_body(tc, posT2, wsh16, wsh32, out2)
    nc.compile()
    return nc


def _core_body(tc, posT2, wsh16, wsh32, out2):
    nc = tc.nc
    from contextlib import ExitStack
    with ExitStack() as ctx:
        dram = ctx.enter_context(tc.tile_pool(name="dram", bufs=1, space="DRAM"))
        cpool = ctx.enter_context(tc.tile_pool(name="consts", bufs=1))
        work = ctx.enter_context(tc.tile_pool(name="work", bufs=3))
        big = ctx.enter_context(tc.tile_pool(name="big", bufs=1))
        persist = ctx.enter_context(tc.tile_pool(name="persist", bufs=1))
        pp = ctx.enter_context(tc.tile_pool(name="ps", bufs=1, space="PSUM"))

        def ps512(shape=None):
            return pp.tile(shape or [128, 512], F32, tag="ps512", name="ps512",
                           bufs=4, padded_shape=[128, 512])

        def ps1024(shape=None):
            return pp.tile(shape or [128, N], F32, tag="ps1024", name="ps1024",
                           bufs=2, padded_shape=[128, N])

        # -------- weight distribution: AllGather the sharded blobs --------
        b16i = dram.tile([1, SH16], BF16, tag="b16i", name="b16i")
        b16o = dram.tile([NCORES, SH16], BF16, tag="b16o", name="b16o")
        b32i = dram.tile([1, SH32], F32, tag="b32i", name="b32i")
        b32o = dram.tile([NCORES, SH32], F32, tag="b32o", name="b32o")
        nc.gpsimd.dma_start(b16i[:], wsh16)
        nc.gpsimd.collective_compute(
            "AllGather", ALU.bypass, replica_groups=[list(range(NCORES))],
            ins=[b16i.opt()], outs=[b16o.opt()])
        nc.gpsimd.dma_start(b32i[:], wsh32)
        nc.gpsimd.collective_compute(
            "AllGather", ALU.bypass, replica_groups=[list(range(NCORES))],
            ins=[b32i.opt()], outs=[b32o.opt()])
        flat16 = b16o[:].rearrange("g w -> (g w)")
        flat32 = b32o[:].rearrange("g w -> (g w)")

        def load16(name):
            p, c, off = L16[name]
            t = cpool.tile([p, c], BF16, tag=name, name=f"c_{name}")
            nc.sync.dma_start(
                out=t[:], in_=flat16[off:off + p * c].rearrange("(p c) -> p c", p=p))
            return t

        def load32(name):
            p, c, off = L32[name]
            t = cpool.tile([p, c], F32, tag=name, name=f"c_{name}")
            nc.sync.dma_start(
                out=t[:], in_=flat32[off:off + p * c].rearrange("(p c) -> p c", p=p))
            return t

        AmB_s = load16("AmB")
        B3_s = load16("B3")
        W1bb_s = load16("W1bb")
        W1cc_s = load16("W1cc")
        W2r2_s = load16("W2r2")
        PmQ2_s = load16("PmQ2")
        Wl_a2_s = load16("Wl_a2")
        Wl_b_s = load16("Wl_b")
        Wm1r_s = load16("Wm1r")
        Wm2r_s = load16("Wm2r")
        Wm3r_s = load16("Wm3r")
        E1r_s = load32("E1r")
        E2r_s = load32("E2r")
        I64st_s = load32("I64st")
        I40_s = load32("I40")
        E1p_s = load32("E1p")
        E2p_s = load32("E2p")
        b1a_s = load32("b1a_c")
        b1bb_s = load32("b1bb")
        b1cc_s = load32("b1cc")
        b2c_s = load32("b2c")
        blT2_s = load32("blT2")
        bm1b_s = load32("bm1b")
        bm2b_s = load32("bm2b")
        bm3T_s = load32("bm3T")

        # -------- structural constants generated on device --------
        iota_s = cpool.tile([128, N], I32, tag="iota", name="iota")
        nc.gpsimd.iota(iota_s[:], [[1, N]], base=0, channel_multiplier=0)
        diag_s = cpool.tile([128, 128], I32, tag="diag", name="diag")
        nc.gpsimd.iota(diag_s[:], [[-1, 128]], base=0, channel_multiplier=1)
        nc.vector.tensor_scalar(out=diag_s[:], in0=diag_s[:], scalar1=0,
                                scalar2=None, op0=ALU.is_equal)
        nc.vector.tensor_scalar(out=diag_s[:], in0=diag_s[:], scalar1=16,
                                scalar2=None, op0=ALU.logical_shift_left)
        # self-index wrapped const: col = pb*160 + e*8 + b, partition q,
        # value = pb*128 + b*16 + q; replicated to 4 groups of 16.
        wrapI_s = cpool.tile([64, 8 * K * 8], I16, tag="wrapI", name="wrapI")
        nc.gpsimd.iota(wrapI_s[0:16, :], [[128, 8], [0, K], [16, 8]], base=0,
                       channel_multiplier=1)
        nc.sync.dma_start(out=wrapI_s[16:32, :], in_=wrapI_s[0:16, :])
        nc.sync.dma_start(out=wrapI_s[32:64, :], in_=wrapI_s[0:32, :])
        ones_s = cpool.tile([1, N], F32, tag="ones", name="ones")
        nc.vector.memset(ones_s[:], 1.0)
        # dummy custom-DVE op: flips compile-time DVE table gen onto the
        # process-level cache path (saves ~0.3 s of host time per call)
        dve0 = cpool.tile([1, 8], F32, tag="dve0", name="dve0")
        nc.vector.memset(dve0[:], 1.0)
        dve1 = cpool.tile([1, 8], F32, tag="dve1", name="dve1")
        nc.vector.reciprocal_approx_fast(out=dve1[:], in_=dve0[:])

        # ---------------- Stage A: pos prep per cloud ----------------
        # tag-sharing plan (persist pool, bufs=1 per tag):
        #   ptab{c}: posT -> preT          aug{c}: A5 -> A66
        #   bug{c}:  B5 -> B66             gtab{c}: vu -> qT
        #   wr{c}:   wrapped1 -> wrapped2  xbuf: x1T -> x1sq -> x2T0
        #   xbuf2: x2T1                    x1Tb: alive to lin1
        posT = [persist.tile([3, N], F32, tag=f"ptab{c}", name=f"posT{c}",
                             padded_shape=[128, N]) for c in range(CPC)]
        A5 = [persist.tile([5, N], F32, tag=f"aug{c}", name=f"A5{c}",
                           padded_shape=[128, N]) for c in range(CPC)]
        B5 = [persist.tile([5, N], F32, tag=f"bug{c}", name=f"B5{c}",
                           padded_shape=[128, N]) for c in range(CPC)]
        posT16 = [work.tile([3, N], BF16, tag=f"pos16_{c}", name=f"posT16_{c}")
                  for c in range(CPC)]
        for c in range(CPC):
            nc.sync.dma_start(out=posT[c][:], in_=posT2[c])
            nc.scalar.activation(posT16[c][:], posT[c][:], AF.Copy)
            p2 = work.tile([3, N], F32, tag="p2", name="p2")
            nc.scalar.activation(p2[:], posT[c][:], AF.Square)
            for h in range(2):
                sl = slice(512 * h, 512 * (h + 1))
                ps5 = ps512([5, 512])
                nc.tensor.matmul(out=ps5[:], lhsT=E1p_s[:],
                                 rhs=posT[c][:, sl],
                                 start=True, stop=False)
                nc.tensor.matmul(out=ps5[:], lhsT=E2p_s[:],
                                 rhs=p2[:, sl],
                                 start=False, stop=True)
                nc.scalar.activation(A5[c][:, sl], ps5[:], AF.Copy)
                nc.scalar.activation(B5[c][:, sl], ps5[:], AF.Copy)
            nc.sync.dma_start(out=A5[c][4:5, :], in_=ones_s[:])
            nc.sync.dma_start(out=B5[c][3:4, :], in_=ones_s[:])

        # vu tables: rows 0-63 = v^T = (x@B)^T ; rows 64-127 = u^T = (x@(A-B)+b1a)^T
        vu = [persist.tile([128, N], F32, tag=f"gtab{c}", name=f"vu{c}")
              for c in range(CPC)]
        for c in range(CPC):
            for h in range(2):
                sl = slice(512 * h, 512 * (h + 1))
                pv = ps512([64, 512])
                nc.tensor.matmul(out=pv[:], lhsT=B3_s[:],
                                 rhs=posT16[c][:, sl], start=True, stop=True)
                nc.scalar.activation(vu[c][0:64, sl], pv[:], AF.Copy)
                pu = ps512([64, 512])
                nc.tensor.matmul(out=pu[:], lhsT=AmB_s[:],
                                 rhs=posT16[c][:, sl], start=True, stop=True)
                nc.scalar.activation(vu[c][64:128, sl], pu[:], AF.Identity,
                                     bias=b1a_s[:])

        # ---------------- Stage B: kNN1 + fold ----------------
        wrapped1 = [persist.tile([128, 8 * K * 8], I16, tag=f"wr{c}",
                                 name=f"wr1{c}") for c in range(CPC)]
        for c in range(CPC):
            idx16_all = work.tile([128, NB * K], I16, tag="idx16", name="idx16")
            for blk in range(NB):
                _knn_block(nc, work, ps1024, A5[c][:, 128 * blk:128 * (blk + 1)],
                           B5[c][:], SCALE1, iota_s, diag_s, idx16_all, blk)
            _fold_idx(nc, idx16_all, wrapped1[c], 2)
            nc.sync.dma_start(out=wrapped1[c][64:128, :], in_=wrapI_s[:])

        # ---------------- Stage D: conv1 ----------------
        x1T = persist.tile([128, N], F32, tag="xbuf", name="x1T")
        for blk in range(NB):
            G = [None, None]
            for c in range(CPC):
                G[c] = big.tile([128, E], F32, tag="gath", name=f"G{c}", bufs=3)
                nc.gpsimd.ap_gather(
                    out_ap=G[c][:], in_ap=vu[c][:],
                    idxs_ap=wrapped1[c][:, 160 * blk:160 * (blk + 1)],
                    channels=128, num_elems=N, d=1, num_idxs=E)
            L3 = big.tile([128, E], F32, tag="L3", name="L3", bufs=2)
            for ch in range(NCH):
                sl = slice(512 * ch, 512 * (ch + 1))
                L12 = work.tile([128, 512], BF16, tag="L12", name="L12")
                for c in range(CPC):
                    ph = ps512([64, 512])
                    nc.tensor.matmul(out=ph[:], lhsT=I64st_s[:],
                                     rhs=G[c][:, sl],
                                     start=True, stop=True)
                    nc.scalar.activation(L12[64 * c:64 * (c + 1), :], ph[:],
                                         AF.Relu)
                p2l = ps512()
                nc.tensor.matmul(out=p2l[:], lhsT=W1bb_s[:],
                                 rhs=L12[:], start=True, stop=True)
                L2 = work.tile([128, 512], BF16, tag="L2", name="L2")
                nc.scalar.activation(L2[:], p2l[:], AF.Relu, bias=b1bb_s[:])
                p3l = ps512()
                nc.tensor.matmul(out=p3l[:], lhsT=W1cc_s[:],
                                 rhs=L2[:], start=True, stop=True)
                nc.scalar.activation(L3[:, sl], p3l[:], AF.Copy)
            nc.vector.tensor_reduce(
                out=x1T[:, 128 * blk:128 * (blk + 1)],
                in_=L3[:].rearrange("c (e p) -> c p e", p=128),
                axis=mybir.AxisListType.X, op=ALU.max)
        x1Tb = persist.tile([128, N], F32, tag="x1Tb", name="x1Tb")
        nc.scalar.activation(x1Tb[:], x1T[:], AF.Identity, bias=b1cc_s[:])
        x1b16 = persist.tile([128, N], BF16, tag="x1b16", name="x1b16")
        nc.scalar.activation(x1b16[:], x1Tb[:], AF.Copy)

        # ---------------- Stage E: kNN2 + fold ----------------
        x1sq = persist.tile([128, N], F32, tag="xbuf", name="x1sq")
        nc.scalar.activation(x1sq[:], x1Tb[:], AF.Square)
        A66 = [persist.tile([66, N], F32, tag=f"aug{c}", name=f"A66{c}",
                            padded_shape=[128, N]) for c in range(CPC)]
        B66 = [persist.tile([66, N], F32, tag=f"bug{c}", name=f"B66{c}",
                            padded_shape=[128, N]) for c in range(CPC)]
        for c in range(CPC):
            half = slice(64 * c, 64 * (c + 1))
            for h in range(2):
                sl = slice(512 * h, 512 * (h + 1))
                p66 = ps512([66, 512])
                nc.tensor.matmul(out=p66[:], lhsT=E1r_s[half, :],
                                 rhs=x1Tb[half, sl],
                                 start=True, stop=False)
                nc.tensor.matmul(out=p66[:], lhsT=E2r_s[half, :],
                                 rhs=x1sq[half, sl],
                                 start=False, stop=True)
                nc.scalar.activation(A66[c][:, sl], p66[:], AF.Copy)
                nc.scalar.activation(B66[c][:, sl], p66[:], AF.Copy)
            nc.sync.dma_start(out=A66[c][65:66, :], in_=ones_s[:])
            nc.sync.dma_start(out=B66[c][64:65, :], in_=ones_s[:])

        wrapped2 = [persist.tile([128, 8 * K * 8], I16, tag=f"wr{c}",
                                 name=f"wr2{c}") for c in range(CPC)]
        for c in range(CPC):
            idx16_all = work.tile([128, NB * K], I16, tag="idx16", name="idx16")
            for blk in range(NB):
                _knn_block(nc, work, ps1024, A66[c][:, 128 * blk:128 * (blk + 1)],
                           B66[c][:], SCALE2, iota_s, diag_s, idx16_all, blk)
            _fold_idx(nc, idx16_all, wrapped2[c], 3)

        # ---------------- Stage F: conv2 ----------------
        x2T = [persist.tile([128, N], F32, tag=("xbuf" if c == 0 else "xbuf2"),
                            name=f"x2T{c}") for c in range(CPC)]
        x2b16 = [persist.tile([128, N], BF16, tag=f"x2b16_{c}", name=f"x2b16_{c}")
                 for c in range(CPC)]
        qT = [persist.tile([128, N], F32, tag=f"gtab{c}", name=f"qT{c}")
              for c in range(CPC)]
        preT = [persist.tile([128, N], F32, tag=f"ptab{c}", name=f"preT{c}")
                for c in range(CPC)]
        for c in range(CPC):
            half = slice(64 * c, 64 * (c + 1))
            for h in range(2):
                sl = slice(512 * h, 512 * (h + 1))
                pq = ps512()
                nc.tensor.matmul(out=pq[:], lhsT=W2r2_s[half, :],
                                 rhs=x1b16[half, sl], start=True, stop=True)
                nc.scalar.activation(qT[c][:, sl], pq[:], AF.Copy)
                ppre = ps512()
                nc.tensor.matmul(out=ppre[:], lhsT=PmQ2_s[half, :],
                                 rhs=x1b16[half, sl], start=True, stop=True)
                nc.scalar.activation(preT[c][:, sl], ppre[:], AF.Identity,
                                     bias=b2c_s[:])
            for blk in range(NB):
                Gq = big.tile([128, E], F32, tag="gath", name="Gq", bufs=3)
                nc.gpsimd.ap_gather(
                    out_ap=Gq[:], in_ap=qT[c][:],
                    idxs_ap=wrapped2[c][:, 160 * blk:160 * (blk + 1)],
                    channels=128, num_elems=N, d=1, num_idxs=E)
                red = work.tile([128, 128], F32, tag="red", name="red")
                nc.vector.tensor_reduce(
                    out=red[:], in_=Gq[:].rearrange("c (e p) -> c p e", p=128),
                    axis=mybir.AxisListType.X, op=ALU.max)
                nc.vector.tensor_tensor(
                    out=x2T[c][:, 128 * blk:128 * (blk + 1)], in0=red[:],
                    in1=preT[c][:, 128 * blk:128 * (blk + 1)], op=ALU.add)
            nc.scalar.activation(x2b16[c][:], x2T[c][:], AF.Copy)

        # ---------------- Stage G: lin1 + global max pool ----------------
        g2 = persist.tile([128, 16], F32, tag="g2", name="g2")
        for c in range(CPC):
            half = slice(64 * c, 64 * (c + 1))
            for cb in range(8):
                cbs = slice(128 * cb, 128 * (cb + 1))
                pl = ps1024()
                for h in range(2):
                    sl = slice(512 * h, 512 * (h + 1))
                    nc.tensor.matmul(out=pl[:, sl],
                                     lhsT=Wl_a2_s[half, cbs],
                                     rhs=x1b16[half, sl],
                                     start=True, stop=False)
                    nc.tensor.matmul(out=pl[:, sl],
                                     lhsT=Wl_b_s[:, cbs],
                                     rhs=x2b16[c][:, sl],
                                     start=False, stop=True)
                nc.vector.tensor_reduce(out=g2[:, 2 * cb + c:2 * cb + c + 1],
                                        in_=pl[:], axis=mybir.AxisListType.X,
                                        op=ALU.max)
        nc.vector.tensor_tensor(out=g2[:], in0=g2[:], in1=blT2_s[:], op=ALU.add)
        g2b = persist.tile([128, 16], BF16, tag="g2b", name="g2b")
        nc.scalar.activation(g2b[:], g2[:], AF.Copy)

        # ---------------- Stage H: head + log_softmax ----------------
        h1s = persist.tile([128, 8], BF16, tag="h1s", name="h1s")
        for m in range(4):
            ph = ps512([128, 2])
            for k in range(8):
                nc.tensor.matmul(out=ph[:],
                                 lhsT=Wm1r_s[:, 512 * k + 128 * m:512 * k + 128 * (m + 1)],
                                 rhs=g2b[:, 2 * k:2 * (k + 1)],
                                 start=(k == 0), stop=(k == 7))
            nc.scalar.activation(h1s[:, 2 * m:2 * (m + 1)], ph[:], AF.Relu,
                                 bias=bm1b_s[:, m:m + 1])
        h2s = persist.tile([128, 4], BF16, tag="h2s", name="h2s")
        for m in range(2):
            ph = ps512([128, 2])
            for j in range(4):
                nc.tensor.matmul(out=ph[:],
                                 lhsT=Wm2r_s[:, 256 * j + 128 * m:256 * j + 128 * (m + 1)],
                                 rhs=h1s[:, 2 * j:2 * (j + 1)],
                                 start=(j == 0), stop=(j == 3))
            nc.scalar.activation(h2s[:, 2 * m:2 * (m + 1)], ph[:], AF.Relu,
                                 bias=bm2b_s[:, m:m + 1])
        plg = ps512([40, 2])
        for j in range(2):
            nc.tensor.matmul(out=plg[:], lhsT=Wm3r_s[:, 40 * j:40 * (j + 1)],
                             rhs=h2s[:, 2 * j:2 * (j + 1)],
                             start=(j == 0), stop=(j == 1))
        lg = persist.tile([40, 2], F32, tag="lg", name="lg")
        nc.scalar.activation(lg[:], plg[:], AF.Identity, bias=bm3T_s[:])
        pt = ps512([2, 40])
        nc.tensor.transpose(out=pt[:], in_=lg[:], identity=I40_s[:])
        lgT = persist.tile([2, 40], F32, tag="lgT", name="lgT")
        nc.scalar.activation(lgT[:], pt[:], AF.Copy)
        negm = persist.tile([2, 1], F32, tag="negm", name="negm")
        nc.vector.tensor_reduce(out=negm[:], in_=lgT[:],
                                axis=mybir.AxisListType.X, op=ALU.max,
                                negate=True)
        t1 = persist.tile([2, 40], F32, tag="t1", name="t1")
        nc.scalar.activation(t1[:], lgT[:], AF.Identity, bias=negm[:])
        ex = persist.tile([2, 40], F32, tag="ex", name="ex")
        nc.scalar.activation(ex[:], lgT[:], AF.Exp, bias=negm[:])
        ssum = persist.tile([2, 1], F32, tag="ssum", name="ssum")
        nc.vector.tensor_reduce(out=ssum[:], in_=ex[:],
                                axis=mybir.AxisListType.X, op=ALU.add)
        lsum = persist.tile([2, 1], F32, tag="lsum", name="lsum")
        nc.scalar.activation(lsum[:], ssum[:], AF.Ln)
        outt = persist.tile([2, 40], F32, tag="outt", name="outt")
        nc.vector.tensor_tensor(out=outt[:], in0=t1[:],
                                in1=lsum[:].to_broadcast([2, 40]),
                                op=ALU.subtract)
        nc.sync.dma_start(out=out2, in_=outt[:])


def _host_prep(inputs):
    """Build the sharded blob inputs and per-core pos inputs."""
    pos = np.asarray(inputs["pos"], dtype=np.float32)
    W1a = np.asarray(inputs["W1a"], np.float32)
    e16, e32 = {}, {}
    e16["AmB"] = W1a[:3] - W1a[3:]
    e16["B3"] = W1a[3:]
    e32["b1a_c"] = np.asarray(inputs["b1a"], np.float32).reshape(64, 1)

    def blockdiag2(w):
        z = np.zeros((128, 128), np.float32)
        z[:64, :64] = w
        z[64:, 64:] = w
        return z

    e16["W1bb"] = blockdiag2(np.asarray(inputs["W1b"], np.float32))
    e32["b1bb"] = np.tile(np.asarray(inputs["b1b"], np.float32), 2).reshape(128, 1)
    e16["W1cc"] = blockdiag2(np.asarray(inputs["W1c"], np.float32))
    e32["b1cc"] = np.tile(np.asarray(inputs["b1c"], np.float32), 2).reshape(128, 1)

    E1 = np.zeros((64, 66), np.float32)
    E1[:, :64] = np.eye(64, dtype=np.float32)
    E2 = np.zeros((64, 66), np.float32)
    E2[:, 64] = -0.5
    E2[:, 65] = -0.5
    e32["E1r"] = np.vstack([E1, E1])
    e32["E2r"] = np.vstack([E2, E2])

    W2 = np.asarray(inputs["W2"], np.float32)
    e16["W2r2"] = np.vstack([W2[64:], W2[64:]])
    e16["PmQ2"] = np.vstack([W2[:64] - W2[64:], W2[:64] - W2[64:]])
    e32["b2c"] = np.asarray(inputs["b2"], np.float32).reshape(128, 1)

    Wl = np.asarray(inputs["Wl"], np.float32)
    e16["Wl_a2"] = np.vstack([Wl[:64], Wl[:64]])
    e16["Wl_b"] = Wl[64:]
    bl = np.asarray(inputs["bl"], np.float32)
    blT = bl.reshape(8, 128).T  # [128, 8]
    e32["blT2"] = np.repeat(blT, 2, axis=1)  # col = cb*2 + cloud

    Wm1 = np.asarray(inputs["Wm1"], np.float32)
    e16["Wm1r"] = Wm1.reshape(8, 128, 512).transpose(1, 0, 2).reshape(128, 8 * 512)
    e32["bm1b"] = np.asarray(inputs["bm1"], np.float32).reshape(4, 128).T
    Wm2 = np.asarray(inputs["Wm2"], np.float32)
    e16["Wm2r"] = Wm2.reshape(4, 128, 256).transpose(1, 0, 2).reshape(128, 4 * 256)
    e32["bm2b"] = np.asarray(inputs["bm2"], np.float32).reshape(2, 128).T
    Wm3 = np.asarray(inputs["Wm3"], np.float32)
    e16["Wm3r"] = Wm3.reshape(2, 128, 40).transpose(1, 0, 2).reshape(128, 2 * 40)
    e32["bm3T"] = np.asarray(inputs["bm3"], np.float32).reshape(40, 1)

    I64 = np.eye(64, dtype=np.float32)
    e32["I64st"] = np.vstack([I64, I64])
    e32["I40"] = np.eye(40, dtype=np.float32)
    E1pm = np.zeros((3, 5), np.float32)
    E1pm[:, :3] = np.eye(3, dtype=np.float32)
    e32["E1p"] = E1pm
    E2pm = np.zeros((3, 5), np.float32)
    E2pm[:, 3] = -0.5
    E2pm[:, 4] = -0.5
    e32["E2p"] = E2pm

    blob16 = np.zeros(NCORES * SH16, dtype=ml_dtypes.bfloat16)
    for name, p, c in LAYOUT16:
        _, _, off = L16[name]
        a = np.asarray(e16[name], np.float32)
        assert a.shape == (p, c), (name, a.shape)
        blob16[off:off + p * c] = a.reshape(-1).astype(ml_dtypes.bfloat16)
    blob32 = np.zeros(NCORES * SH32, dtype=np.float32)
    for name, p, c in LAYOUT32:
        _, _, off = L32[name]
        a = np.asarray(e32[name], np.float32)
        assert a.shape == (p, c), (name, a.shape)
        blob32[off:off + p * c] = a.reshape(-1)

    per_core = []
    for core in range(NCORES):
        per_core.append({
            "posT2": np.ascontiguousarray(
                pos[CPC * core:CPC * (core + 1)].transpose(0, 2, 1)),
            "wsh16": blob16[core * SH16:(core + 1) * SH16][None, :],
            "wsh32": blob32[core * SH32:(core + 1) * SH32][None, :],
        })
    return per_core


@lru_cache(maxsize=1)
def _get_program():
    return build_program()


def kernel(**inputs):
    nc = _get_program()
    in_maps = _host_prep(inputs)
    res = run_bass_kernel_spmd(nc, in_maps, core_ids=list(range(NCORES)))
    outs = [res.results[i]["out2"] for i in range(NCORES)]
    return np.concatenate(outs, axis=0).astype(np.float32)


if __name__ == "__main__":
    pass


# revision 5
# speedup vs baseline: 2.3299x; 2.3299x over previous
"""DGCNN-style point-cloud classifier on 8 Trainium2 NeuronCores.

Data-parallel over the B=16 point-cloud axis: each of the 8 cores processes 2
clouds end-to-end (kNN -> EdgeConv1 -> kNN -> EdgeConv2 -> lin1 -> global max
pool -> head -> log_softmax).

Host-dispatch-optimized revision: the per-call wall clock is dominated by
shipping inputs over the PJRT tunnel, so the weights travel as ONE bf16 blob
sharded 1/8th per core and AllGather-ed on device (plus a small f32 blob for
biases/structural matrices), and the purely structural constants (iota, the
kNN diagonal mask, the self-index gather table) are generated on device with
iota/memset instructions.  Total upload: ~2.3 MB vs ~39 MB for the naive
per-core replication.  A dummy custom-DVE op keeps the per-call DVE table
generation on the process-level cache path.

Device-side ideas (unchanged from the baseline):
  * kNN top-20 per point via packed int32 keys (2^30 - d*S | neighbor index in
    the low 10 bits) extracted with DVE Max8 + MatchReplace (3+2 passes).
  * Neighbor gathers with GPSIMD ap_gather in a feature-major layout, which is
    exactly the transposed layout TensorE wants for the per-edge MLP.
  * EdgeConv2's single linear layer folds through the max-aggregation:
    out_i = pre_i + max_j q_j, so no per-edge GEMM at all.
  * GEMM weights are kept in bf16 (activations converted at layer inputs);
    the kNN distance pipeline stays fully fp32.
"""

import sys
import numpy as np
from functools import lru_cache

for _p in ("/opt/trn_rl_repo", "/root/.axon_site/_ro/trn_rl_repo"):
    if _p not in sys.path:
        sys.path.insert(0, _p)

import ml_dtypes
import concourse.bass as bass
import concourse.bacc as bacc
import concourse.mybir as mybir
import concourse.tile as tile
from concourse.bass_utils import run_bass_kernel_spmd

# Persistent PJRT compilation cache: the SPMD runner builds a fresh jit
# closure per call, so without this every kernel() call re-runs the full
# HLO->walrus->NEFF pipeline (~150 ms) even though the program is identical.
try:
    import os as _os
    import tempfile as _tempfile
    import jax as _jax
    if _jax.config.jax_compilation_cache_dir is None:
        _jax.config.update(
            "jax_compilation_cache_dir",
            _os.path.join(_tempfile.gettempdir(), "jax_pjrt_ccache"))
        _jax.config.update("jax_persistent_cache_min_entry_size_bytes", 0)
        _jax.config.update("jax_persistent_cache_min_compile_time_secs", 0.0)
except Exception:
    pass

AF = mybir.ActivationFunctionType
ALU = mybir.AluOpType
DT = mybir.dt
F32 = DT.float32
BF16 = DT.bfloat16
I32 = DT.int32
I16 = DT.int16

N = 1024          # points per cloud
K = 20            # neighbors
NCORES = 8
CPC = 2           # clouds per core
NB = 8            # point blocks of 128 per cloud
E = K * 128       # edges per point block (2560)
NCH = 5           # 512-col chunks per point block of edges

SCALE1 = float(1 << 24)   # key scale for kNN1 (d range 127, resolution 2^-14)
SCALE2 = float(1 << 20)   # key scale for kNN2 (d range 2040, resolution 2^-10)
BIAS30 = float(1 << 30)

# ---- packed-blob layouts: (name, partitions, cols); flattened p-major ----
LAYOUT16 = [
    ("W1bb", 128, 128), ("W1cc", 128, 128), ("W2r2", 128, 128),
    ("PmQ2", 128, 128), ("Wl_a2", 128, 1024), ("Wl_b", 128, 1024),
    ("Wm1r", 128, 4096), ("Wm2r", 128, 1024), ("Wm3r", 128, 80),
    ("AmB", 3, 64), ("B3", 3, 64),
]
LAYOUT32 = [
    ("E1r", 128, 66), ("E2r", 128, 66), ("I64st", 128, 64), ("I40", 40, 40),
    ("E1p", 3, 5), ("E2p", 3, 5), ("b1a_c", 64, 1), ("b1bb", 128, 1),
    ("b1cc", 128, 1), ("b2c", 128, 1), ("blT2", 128, 16), ("bm1b", 128, 4),
    ("bm2b", 128, 2), ("bm3T", 40, 1),
]


def _offsets(layout):
    d, off = {}, 0
    for name, p, c in layout:
        d[name] = (p, c, off)
        off += p * c
    return d, off


L16, TOT16 = _offsets(LAYOUT16)
L32, TOT32 = _offsets(LAYOUT32)
SH16 = -(-TOT16 // 8)          # per-core shard elems (bf16)
SH32 = -(-TOT32 // 8)          # per-core shard elems (f32)


def _knn_block(nc, pool, psum_alloc, lhsT_A, rhs_B, scale, iota2d, diag2048,
               idx16_all, blk):
    """Top-20 neighbor indices for one 128-point block.

    lhsT_A: [Kc x 128] block slice of the augmented A operand.
    rhs_B:  [Kc x 1024] augmented B operand. psum = A.T@B = -d/2 per pair.
    Writes int16 indices into idx16_all[:, 20*blk : 20*(blk+1)].
    """
    ps = psum_alloc()
    nc.tensor.matmul(out=ps[:, 0:512], lhsT=lhsT_A,
                     rhs=rhs_B[:, 0:512], start=True, stop=True)
    nc.tensor.matmul(out=ps[:, 512:1024], lhsT=lhsT_A,
                     rhs=rhs_B[:, 512:1024], start=True, stop=True)
    keys = pool.tile([128, N], I32, tag="keys", name="keys")
    nc.scalar.activation(keys[:], ps[:], AF.Copy, bias=BIAS30, scale=scale)
    # clear low 10 bits, boost the diagonal (self) above everything, add index
    nc.vector.tensor_scalar(out=keys[:], in0=keys[:], scalar1=-1024,
                            scalar2=None, op0=ALU.bitwise_and)
    nc.vector.tensor_tensor(out=keys[:, 128 * blk:128 * (blk + 1)],
                            in0=keys[:, 128 * blk:128 * (blk + 1)],
                            in1=diag2048[:], op=ALU.add)
    nc.vector.tensor_tensor(out=keys[:], in0=keys[:], in1=iota2d[:],
                            op=ALU.bitwise_or)
    kf = keys[:].bitcast(F32)
    top = pool.tile([128, 24], F32, tag="top24", name="top24")
    nc.vector.max(out=top[:, 0:8], in_=kf)
    nc.vector.match_replace(out=kf, in_to_replace=top[:, 0:8], in_values=kf,
                            imm_value=0.0)
    nc.vector.max(out=top[:, 8:16], in_=kf)
    nc.vector.match_replace(out=kf, in_to_replace=top[:, 8:16], in_values=kf,
                            imm_value=0.0)
    nc.vector.max(out=top[:, 16:24], in_=kf)
    # col 0 is self; neighbor indices are the low 10 bits of cols 1..20
    idxs = pool.tile([128, K], I32, tag="idx32", name="idx32")
    nc.vector.tensor_scalar(out=idxs[:], in0=top[:, 1:21].bitcast(I32),
                            scalar1=1023, scalar2=None, op0=ALU.bitwise_and)
    nc.vector.tensor_copy(out=idx16_all[:, K * blk:K * (blk + 1)], in_=idxs[:])


def _fold_idx(nc, idx16_all, wrapped, ngroups_log2):
    """[128 x 160] per-point indices -> ap_gather wrapped layout [16 x 1280],
    then replicate across partition groups by doubling."""
    for b in range(8):
        src = idx16_all[16 * b:16 * (b + 1), :].rearrange("q (pb e) -> q pb e", e=K)
        dst = wrapped[0:16, :].rearrange("q (pb e b) -> q pb e b", e=K, b=8)[:, :, :, b]
        nc.sync.dma_start(out=dst, in_=src)
    for i in range(ngroups_log2):
        w = 16 << i
        nc.sync.dma_start(out=wrapped[w:2 * w, :], in_=wrapped[0:w, :])


def build_program():
    nc = bacc.Bacc("TRN2", target_bir_lowering=False, debug=False)

    def inp(name, shape, dtype=F32):
        return nc.dram_tensor(name, list(shape), dtype, kind="ExternalInput").ap()

    posT2 = inp("posT2", (CPC, 3, N))
    wsh16 = inp("wsh16", (1, SH16), BF16)
    wsh32 = inp("wsh32", (1, SH32), F32)
    out2 = nc.dram_tensor("out2", [CPC, 40], F32, kind="ExternalOutput").ap()

    with tile.TileContext(nc) as tc:
        _core_body(tc, posT2, wsh16, wsh32, out2)
    nc.compile()
    return nc


def _core_body(tc, posT2, wsh16, wsh32, out2):
    nc = tc.nc
    from contextlib import ExitStack
    with ExitStack() as ctx:
        dram = ctx.enter_context(tc.tile_pool(name="dram", bufs=1, space="DRAM"))
        cpool = ctx.enter_context(tc.tile_pool(name="consts", bufs=1))
        work = ctx.enter_context(tc.tile_pool(name="work", bufs=3))
        big = ctx.enter_context(tc.tile_pool(name="big", bufs=1))
        persist = ctx.enter_context(tc.tile_pool(name="persist", bufs=1))
        pp = ctx.enter_context(tc.tile_pool(name="ps", bufs=1, space="PSUM"))

        def ps512(shape=None):
            return pp.tile(shape or [128, 512], F32, tag="ps512", name="ps512",
                           bufs=4, padded_shape=[128, 512])

        def ps1024(shape=None):
            return pp.tile(shape or [128, N], F32, tag="ps1024", name="ps1024",
                           bufs=2, padded_shape=[128, N])

        # -------- weight distribution: AllGather the sharded blobs --------
        b16i = dram.tile([1, SH16], BF16, tag="b16i", name="b16i")
        b16o = dram.tile([NCORES, SH16], BF16, tag="b16o", name="b16o")
        b32i = dram.tile([1, SH32], F32, tag="b32i", name="b32i")
        b32o = dram.tile([NCORES, SH32], F32, tag="b32o", name="b32o")
        nc.gpsimd.dma_start(b16i[:], wsh16)
        nc.gpsimd.collective_compute(
            "AllGather", ALU.bypass, replica_groups=[list(range(NCORES))],
            ins=[b16i.opt()], outs=[b16o.opt()])
        nc.gpsimd.dma_start(b32i[:], wsh32)
        nc.gpsimd.collective_compute(
            "AllGather", ALU.bypass, replica_groups=[list(range(NCORES))],
            ins=[b32i.opt()], outs=[b32o.opt()])
        flat16 = b16o[:].rearrange("g w -> (g w)")
        flat32 = b32o[:].rearrange("g w -> (g w)")

        def load16(name):
            p, c, off = L16[name]
            t = cpool.tile([p, c], BF16, tag=name, name=f"c_{name}")
            nc.sync.dma_start(
                out=t[:], in_=flat16[off:off + p * c].rearrange("(p c) -> p c", p=p))
            return t

        def load32(name):
            p, c, off = L32[name]
            t = cpool.tile([p, c], F32, tag=name, name=f"c_{name}")
            nc.sync.dma_start(
                out=t[:], in_=flat32[off:off + p * c].rearrange("(p c) -> p c", p=p))
            return t

        AmB_s = load16("AmB")
        B3_s = load16("B3")
        W1bb_s = load16("W1bb")
        W1cc_s = load16("W1cc")
        W2r2_s = load16("W2r2")
        PmQ2_s = load16("PmQ2")
        Wl_a2_s = load16("Wl_a2")
        Wl_b_s = load16("Wl_b")
        Wm1r_s = load16("Wm1r")
        Wm2r_s = load16("Wm2r")
        Wm3r_s = load16("Wm3r")
        E1r_s = load32("E1r")
        E2r_s = load32("E2r")
        I64st_s = load32("I64st")
        I40_s = load32("I40")
        E1p_s = load32("E1p")
        E2p_s = load32("E2p")
        b1a_s = load32("b1a_c")
        b1bb_s = load32("b1bb")
        b1cc_s = load32("b1cc")
        b2c_s = load32("b2c")
        blT2_s = load32("blT2")
        bm1b_s = load32("bm1b")
        bm2b_s = load32("bm2b")
        bm3T_s = load32("bm3T")

        # -------- structural constants generated on device --------
        iota_s = cpool.tile([128, N], I32, tag="iota", name="iota")
        nc.gpsimd.iota(iota_s[:], [[1, N]], base=0, channel_multiplier=0)
        diag_s = cpool.tile([128, 128], I32, tag="diag", name="diag")
        nc.gpsimd.iota(diag_s[:], [[-1, 128]], base=0, channel_multiplier=1)
        nc.vector.tensor_scalar(out=diag_s[:], in0=diag_s[:], scalar1=0,
                                scalar2=None, op0=ALU.is_equal)
        nc.vector.tensor_scalar(out=diag_s[:], in0=diag_s[:], scalar1=16,
                                scalar2=None, op0=ALU.logical_shift_left)
        # self-index wrapped const: col = pb*160 + e*8 + b, partition q,
        # value = pb*128 + b*16 + q; replicated to 4 groups of 16.
        wrapI_s = cpool.tile([64, 8 * K * 8], I16, tag="wrapI", name="wrapI")
        nc.gpsimd.iota(wrapI_s[0:16, :], [[128, 8], [0, K], [16, 8]], base=0,
                       channel_multiplier=1)
        nc.sync.dma_start(out=wrapI_s[16:32, :], in_=wrapI_s[0:16, :])
        nc.sync.dma_start(out=wrapI_s[32:64, :], in_=wrapI_s[0:32, :])
        ones_s = cpool.tile([1, N], F32, tag="ones", name="ones")
        nc.vector.memset(ones_s[:], 1.0)
        # dummy custom-DVE op: flips compile-time DVE table gen onto the
        # process-level cache path (saves ~0.3 s of host time per call)
        dve0 = cpool.tile([1, 8], F32, tag="dve0", name="dve0")
        nc.vector.memset(dve0[:], 1.0)
        dve1 = cpool.tile([1, 8], F32, tag="dve1", name="dve1")
        nc.vector.reciprocal_approx_fast(out=dve1[:], in_=dve0[:])

        # ---------------- Stage A: pos prep per cloud ----------------
        # tag-sharing plan (persist pool, bufs=1 per tag):
        #   ptab{c}: posT -> preT          aug{c}: A5 -> A66
        #   bug{c}:  B5 -> B66             gtab{c}: vu -> qT
        #   wr{c}:   wrapped1 -> wrapped2  xbuf: x1T -> x1sq -> x2T0
        #   xbuf2: x2T1                    x1Tb: alive to lin1
        posT = [persist.tile([3, N], F32, tag=f"ptab{c}", name=f"posT{c}",
                             padded_shape=[128, N]) for c in range(CPC)]
        A5 = [persist.tile([5, N], F32, tag=f"aug{c}", name=f"A5{c}",
                           padded_shape=[128, N]) for c in range(CPC)]
        B5 = [persist.tile([5, N], F32, tag=f"bug{c}", name=f"B5{c}",
                           padded_shape=[128, N]) for c in range(CPC)]
        posT16 = [work.tile([3, N], BF16, tag=f"pos16_{c}", name=f"posT16_{c}")
                  for c in range(CPC)]
        for c in range(CPC):
            nc.sync.dma_start(out=posT[c][:], in_=posT2[c])
            nc.scalar.activation(posT16[c][:], posT[c][:], AF.Copy)
            p2 = work.tile([3, N], F32, tag="p2", name="p2")
            nc.scalar.activation(p2[:], posT[c][:], AF.Square)
            for h in range(2):
                sl = slice(512 * h, 512 * (h + 1))
                ps5 = ps512([5, 512])
                nc.tensor.matmul(out=ps5[:], lhsT=E1p_s[:],
                                 rhs=posT[c][:, sl],
                                 start=True, stop=False)
                nc.tensor.matmul(out=ps5[:], lhsT=E2p_s[:],
                                 rhs=p2[:, sl],
                                 start=False, stop=True)
                nc.scalar.activation(A5[c][:, sl], ps5[:], AF.Copy)
                nc.scalar.activation(B5[c][:, sl], ps5[:], AF.Copy)
            nc.sync.dma_start(out=A5[c][4:5, :], in_=ones_s[:])
            nc.sync.dma_start(out=B5[c][3:4, :], in_=ones_s[:])

        # vu tables: rows 0-63 = v^T = (x@B)^T ; rows 64-127 = u^T = (x@(A-B)+b1a)^T
        vu = [persist.tile([128, N], F32, tag=f"gtab{c}", name=f"vu{c}")
              for c in range(CPC)]
        for c in range(CPC):
            for h in range(2):
                sl = slice(512 * h, 512 * (h + 1))
                pv = ps512([64, 512])
                nc.tensor.matmul(out=pv[:], lhsT=B3_s[:],
                                 rhs=posT16[c][:, sl], start=True, stop=True)
                nc.scalar.activation(vu[c][0:64, sl], pv[:], AF.Copy)
                pu = ps512([64, 512])
                nc.tensor.matmul(out=pu[:], lhsT=AmB_s[:],
                                 rhs=posT16[c][:, sl], start=True, stop=True)
                nc.scalar.activation(vu[c][64:128, sl], pu[:], AF.Identity,
                                     bias=b1a_s[:])

        # ---------------- Stage B: kNN1 + fold ----------------
        wrapped1 = [persist.tile([128, 8 * K * 8], I16, tag=f"wr{c}",
                                 name=f"wr1{c}") for c in range(CPC)]
        for c in range(CPC):
            idx16_all = work.tile([128, NB * K], I16, tag="idx16", name="idx16")
            for blk in range(NB):
                _knn_block(nc, work, ps1024, A5[c][:, 128 * blk:128 * (blk + 1)],
                           B5[c][:], SCALE1, iota_s, diag_s, idx16_all, blk)
            _fold_idx(nc, idx16_all, wrapped1[c], 2)
            nc.sync.dma_start(out=wrapped1[c][64:128, :], in_=wrapI_s[:])

        # ---------------- Stage D: conv1 ----------------
        x1T = persist.tile([128, N], F32, tag="xbuf", name="x1T")
        for blk in range(NB):
            G = [None, None]
            for c in range(CPC):
                G[c] = big.tile([128, E], F32, tag="gath", name=f"G{c}", bufs=3)
                nc.gpsimd.ap_gather(
                    out_ap=G[c][:], in_ap=vu[c][:],
                    idxs_ap=wrapped1[c][:, 160 * blk:160 * (blk + 1)],
                    channels=128, num_elems=N, d=1, num_idxs=E)
            L3 = big.tile([128, E], F32, tag="L3", name="L3", bufs=2)
            for ch in range(NCH):
                sl = slice(512 * ch, 512 * (ch + 1))
                L12 = work.tile([128, 512], BF16, tag="L12", name="L12")
                for c in range(CPC):
                    ph = ps512([64, 512])
                    nc.tensor.matmul(out=ph[:], lhsT=I64st_s[:],
                                     rhs=G[c][:, sl],
                                     start=True, stop=True)
                    nc.scalar.activation(L12[64 * c:64 * (c + 1), :], ph[:],
                                         AF.Relu)
                p2l = ps512()
                nc.tensor.matmul(out=p2l[:], lhsT=W1bb_s[:],
                                 rhs=L12[:], start=True, stop=True)
                L2 = work.tile([128, 512], BF16, tag="L2", name="L2")
                nc.scalar.activation(L2[:], p2l[:], AF.Relu, bias=b1bb_s[:])
                p3l = ps512()
                nc.tensor.matmul(out=p3l[:], lhsT=W1cc_s[:],
                                 rhs=L2[:], start=True, stop=True)
                nc.scalar.activation(L3[:, sl], p3l[:], AF.Copy)
            nc.vector.tensor_reduce(
                out=x1T[:, 128 * blk:128 * (blk + 1)],
                in_=L3[:].rearrange("c (e p) -> c p e", p=128),
                axis=mybir.AxisListType.X, op=ALU.max)
        x1Tb = persist.tile([128, N], F32, tag="x1Tb", name="x1Tb")
        nc.scalar.activation(x1Tb[:], x1T[:], AF.Identity, bias=b1cc_s[:])
        x1b16 = persist.tile([128, N], BF16, tag="x1b16", name="x1b16")
        nc.scalar.activation(x1b16[:], x1Tb[:], AF.Copy)

        # ---------------- Stage E: kNN2 + fold ----------------
        x1sq = persist.tile([128, N], F32, tag="xbuf", name="x1sq")
        nc.scalar.activation(x1sq[:], x1Tb[:], AF.Square)
        A66 = [persist.tile([66, N], F32, tag=f"aug{c}", name=f"A66{c}",
                            padded_shape=[128, N]) for c in range(CPC)]
        B66 = [persist.tile([66, N], F32, tag=f"bug{c}", name=f"B66{c}",
                            padded_shape=[128, N]) for c in range(CPC)]
        for c in range(CPC):
            half = slice(64 * c, 64 * (c + 1))
            for h in range(2):
                sl = slice(512 * h, 512 * (h + 1))
                p66 = ps512([66, 512])
                nc.tensor.matmul(out=p66[:], lhsT=E1r_s[half, :],
                                 rhs=x1Tb[half, sl],
                                 start=True, stop=False)
                nc.tensor.matmul(out=p66[:], lhsT=E2r_s[half, :],
                                 rhs=x1sq[half, sl],
                                 start=False, stop=True)
                nc.scalar.activation(A66[c][:, sl], p66[:], AF.Copy)
                nc.scalar.activation(B66[c][:, sl], p66[:], AF.Copy)
            nc.sync.dma_start(out=A66[c][65:66, :], in_=ones_s[:])
            nc.sync.dma_start(out=B66[c][64:65, :], in_=ones_s[:])

        wrapped2 = [persist.tile([128, 8 * K * 8], I16, tag=f"wr{c}",
                                 name=f"wr2{c}") for c in range(CPC)]
        for c in range(CPC):
            idx16_all = work.tile([128, NB * K], I16, tag="idx16", name="idx16")
            for blk in range(NB):
                _knn_block(nc, work, ps1024, A66[c][:, 128 * blk:128 * (blk + 1)],
                           B66[c][:], SCALE2, iota_s, diag_s, idx16_all, blk)
            _fold_idx(nc, idx16_all, wrapped2[c], 3)

        # ---------------- Stage F: conv2 ----------------
        x2T = [persist.tile([128, N], F32, tag=("xbuf" if c == 0 else "xbuf2"),
                            name=f"x2T{c}") for c in range(CPC)]
        x2b16 = [persist.tile([128, N], BF16, tag=f"x2b16_{c}", name=f"x2b16_{c}")
                 for c in range(CPC)]
        qT = [persist.tile([128, N], F32, tag=f"gtab{c}", name=f"qT{c}")
              for c in range(CPC)]
        preT = [persist.tile([128, N], F32, tag=f"ptab{c}", name=f"preT{c}")
                for c in range(CPC)]
        for c in range(CPC):
            half = slice(64 * c, 64 * (c + 1))
            for h in range(2):
                sl = slice(512 * h, 512 * (h + 1))
                pq = ps512()
                nc.tensor.matmul(out=pq[:], lhsT=W2r2_s[half, :],
                                 rhs=x1b16[half, sl], start=True, stop=True)
                nc.scalar.activation(qT[c][:, sl], pq[:], AF.Copy)
                ppre = ps512()
                nc.tensor.matmul(out=ppre[:], lhsT=PmQ2_s[half, :],
                                 rhs=x1b16[half, sl], start=True, stop=True)
                nc.scalar.activation(preT[c][:, sl], ppre[:], AF.Identity,
                                     bias=b2c_s[:])
            for blk in range(NB):
                Gq = big.tile([128, E], F32, tag="gath", name="Gq", bufs=3)
                nc.gpsimd.ap_gather(
                    out_ap=Gq[:], in_ap=qT[c][:],
                    idxs_ap=wrapped2[c][:, 160 * blk:160 * (blk + 1)],
                    channels=128, num_elems=N, d=1, num_idxs=E)
                red = work.tile([128, 128], F32, tag="red", name="red")
                nc.vector.tensor_reduce(
                    out=red[:], in_=Gq[:].rearrange("c (e p) -> c p e", p=128),
                    axis=mybir.AxisListType.X, op=ALU.max)
                nc.vector.tensor_tensor(
                    out=x2T[c][:, 128 * blk:128 * (blk + 1)], in0=red[:],
                    in1=preT[c][:, 128 * blk:128 * (blk + 1)], op=ALU.add)
            nc.scalar.activation(x2b16[c][:], x2T[c][:], AF.Copy)

        # ---------------- Stage G: lin1 + global max pool ----------------
        g2 = persist.tile([128, 16], F32, tag="g2", name="g2")
        for c in range(CPC):
            half = slice(64 * c, 64 * (c + 1))
            for cb in range(8):
                cbs = slice(128 * cb, 128 * (cb + 1))
                pl = ps1024()
                for h in range(2):
                    sl = slice(512 * h, 512 * (h + 1))
                    nc.tensor.matmul(out=pl[:, sl],
                                     lhsT=Wl_a2_s[half, cbs],
                                     rhs=x1b16[half, sl],
                                     start=True, stop=False)
                    nc.tensor.matmul(out=pl[:, sl],
                                     lhsT=Wl_b_s[:, cbs],
                                     rhs=x2b16[c][:, sl],
                                     start=False, stop=True)
                nc.vector.tensor_reduce(out=g2[:, 2 * cb + c:2 * cb + c + 1],
                                        in_=pl[:], axis=mybir.AxisListType.X,
                                        op=ALU.max)
        nc.vector.tensor_tensor(out=g2[:], in0=g2[:], in1=blT2_s[:], op=ALU.add)
        g2b = persist.tile([128, 16], BF16, tag="g2b", name="g2b")
        nc.scalar.activation(g2b[:], g2[:], AF.Copy)

        # ---------------- Stage H: head + log_softmax ----------------
        h1s = persist.tile([128, 8], BF16, tag="h1s", name="h1s")
        for m in range(4):
            ph = ps512([128, 2])
            for k in range(8):
                nc.tensor.matmul(out=ph[:],
                                 lhsT=Wm1r_s[:, 512 * k + 128 * m:512 * k + 128 * (m + 1)],
                                 rhs=g2b[:, 2 * k:2 * (k + 1)],
                                 start=(k == 0), stop=(k == 7))
            nc.scalar.activation(h1s[:, 2 * m:2 * (m + 1)], ph[:], AF.Relu,
                                 bias=bm1b_s[:, m:m + 1])
        h2s = persist.tile([128, 4], BF16, tag="h2s", name="h2s")
        for m in range(2):
            ph = ps512([128, 2])
            for j in range(4):
                nc.tensor.matmul(out=ph[:],
                                 lhsT=Wm2r_s[:, 256 * j + 128 * m:256 * j + 128 * (m + 1)],
                                 rhs=h1s[:, 2 * j:2 * (j + 1)],
                                 start=(j == 0), stop=(j == 3))
            nc.scalar.activation(h2s[:, 2 * m:2 * (m + 1)], ph[:], AF.Relu,
                                 bias=bm2b_s[:, m:m + 1])
        plg = ps512([40, 2])
        for j in range(2):
            nc.tensor.matmul(out=plg[:], lhsT=Wm3r_s[:, 40 * j:40 * (j + 1)],
                             rhs=h2s[:, 2 * j:2 * (j + 1)],
                             start=(j == 0), stop=(j == 1))
        lg = persist.tile([40, 2], F32, tag="lg", name="lg")
        nc.scalar.activation(lg[:], plg[:], AF.Identity, bias=bm3T_s[:])
        pt = ps512([2, 40])
        nc.tensor.transpose(out=pt[:], in_=lg[:], identity=I40_s[:])
        lgT = persist.tile([2, 40], F32, tag="lgT", name="lgT")
        nc.scalar.activation(lgT[:], pt[:], AF.Copy)
        negm = persist.tile([2, 1], F32, tag="negm", name="negm")
        nc.vector.tensor_reduce(out=negm[:], in_=lgT[:],
                                axis=mybir.AxisListType.X, op=ALU.max,
                                negate=True)
        t1 = persist.tile([2, 40], F32, tag="t1", name="t1")
        nc.scalar.activation(t1[:], lgT[:], AF.Identity, bias=negm[:])
        ex = persist.tile([2, 40], F32, tag="ex", name="ex")
        nc.scalar.activation(ex[:], lgT[:], AF.Exp, bias=negm[:])
        ssum = persist.tile([2, 1], F32, tag="ssum", name="ssum")
        nc.vector.tensor_reduce(out=ssum[:], in_=ex[:],
                                axis=mybir.AxisListType.X, op=ALU.add)
        lsum = persist.tile([2, 1], F32, tag="lsum", name="lsum")
        nc.scalar.activation(lsum[:], ssum[:], AF.Ln)
        outt = persist.tile([2, 40], F32, tag="outt", name="outt")
        nc.vector.tensor_tensor(out=outt[:], in0=t1[:],
                                in1=lsum[:].to_broadcast([2, 40]),
                                op=ALU.subtract)
        nc.sync.dma_start(out=out2, in_=outt[:])


def _host_prep(inputs):
    """Build the sharded blob inputs and per-core pos inputs."""
    pos = np.asarray(inputs["pos"], dtype=np.float32)
    W1a = np.asarray(inputs["W1a"], np.float32)
    e16, e32 = {}, {}
    e16["AmB"] = W1a[:3] - W1a[3:]
    e16["B3"] = W1a[3:]
    e32["b1a_c"] = np.asarray(inputs["b1a"], np.float32).reshape(64, 1)

    def blockdiag2(w):
        z = np.zeros((128, 128), np.float32)
        z[:64, :64] = w
        z[64:, 64:] = w
        return z

    e16["W1bb"] = blockdiag2(np.asarray(inputs["W1b"], np.float32))
    e32["b1bb"] = np.tile(np.asarray(inputs["b1b"], np.float32), 2).reshape(128, 1)
    e16["W1cc"] = blockdiag2(np.asarray(inputs["W1c"], np.float32))
    e32["b1cc"] = np.tile(np.asarray(inputs["b1c"], np.float32), 2).reshape(128, 1)

    E1 = np.zeros((64, 66), np.float32)
    E1[:, :64] = np.eye(64, dtype=np.float32)
    E2 = np.zeros((64, 66), np.float32)
    E2[:, 64] = -0.5
    E2[:, 65] = -0.5
    e32["E1r"] = np.vstack([E1, E1])
    e32["E2r"] = np.vstack([E2, E2])

    W2 = np.asarray(inputs["W2"], np.float32)
    e16["W2r2"] = np.vstack([W2[64:], W2[64:]])
    e16["PmQ2"] = np.vstack([W2[:64] - W2[64:], W2[:64] - W2[64:]])
    e32["b2c"] = np.asarray(inputs["b2"], np.float32).reshape(128, 1)

    Wl = np.asarray(inputs["Wl"], np.float32)
    e16["Wl_a2"] = np.vstack([Wl[:64], Wl[:64]])
    e16["Wl_b"] = Wl[64:]
    bl = np.asarray(inputs["bl"], np.float32)
    blT = bl.reshape(8, 128).T  # [128, 8]
    e32["blT2"] = np.repeat(blT, 2, axis=1)  # col = cb*2 + cloud

    Wm1 = np.asarray(inputs["Wm1"], np.float32)
    e16["Wm1r"] = Wm1.reshape(8, 128, 512).transpose(1, 0, 2).reshape(128, 8 * 512)
    e32["bm1b"] = np.asarray(inputs["bm1"], np.float32).reshape(4, 128).T
    Wm2 = np.asarray(inputs["Wm2"], np.float32)
    e16["Wm2r"] = Wm2.reshape(4, 128, 256).transpose(1, 0, 2).reshape(128, 4 * 256)
    e32["bm2b"] = np.asarray(inputs["bm2"], np.float32).reshape(2, 128).T
    Wm3 = np.asarray(inputs["Wm3"], np.float32)
    e16["Wm3r"] = Wm3.reshape(2, 128, 40).transpose(1, 0, 2).reshape(128, 2 * 40)
    e32["bm3T"] = np.asarray(inputs["bm3"], np.float32).reshape(40, 1)

    I64 = np.eye(64, dtype=np.float32)
    e32["I64st"] = np.vstack([I64, I64])
    e32["I40"] = np.eye(40, dtype=np.float32)
    E1pm = np.zeros((3, 5), np.float32)
    E1pm[:, :3] = np.eye(3, dtype=np.float32)
    e32["E1p"] = E1pm
    E2pm = np.zeros((3, 5), np.float32)
    E2pm[:, 3] = -0.5
    E2pm[:, 4] = -0.5
    e32["E2p"] = E2pm

    blob16 = np.zeros(NCORES * SH16, dtype=ml_dtypes.bfloat16)
    for name, p, c in LAYOUT16:
        _, _, off = L16[name]
        a = np.asarray(e16[name], np.float32)
        assert a.shape == (p, c), (name, a.shape)
        blob16[off:off + p * c] = a.reshape(-1).astype(ml_dtypes.bfloat16)
    blob32 = np.zeros(NCORES * SH32, dtype=np.float32)
    for name, p, c in LAYOUT32:
        _, _, off = L32[name]
        a = np.asarray(e32[name], np.float32)
        assert a.shape == (p, c), (name, a.shape)
        blob32[off:off + p * c] = a.reshape(-1)

    per_core = []
    for core in range(NCORES):
        per_core.append({
            "posT2": np.ascontiguousarray(
                pos[CPC * core:CPC * (core + 1)].transpose(0, 2, 1)),
            "wsh16": blob16[core * SH16:(core + 1) * SH16][None, :],
            "wsh32": blob32[core * SH32:(core + 1) * SH32][None, :],
        })
    return per_core


@lru_cache(maxsize=1)
def _get_program():
    return build_program()


def kernel(**inputs):
    nc = _get_program()
    in_maps = _host_prep(inputs)
    res = run_bass_kernel_spmd(nc, in_maps, core_ids=list(range(NCORES)))
    outs = [res.results[i]["out2"] for i in range(NCORES)]
    return np.concatenate(outs, axis=0).astype(np.float32)


if __name__ == "__main__":
    pass
